# revision 15
# baseline (speedup 1.0000x reference)
"""GMM log-likelihood kernel for Trainium2 (Bass/Tile), 8-core data-parallel.

v3 design. Math (host precompute in f64):
  B_k = L_k^{-1},  w_k = B_k^T B_k mu_k
  wlp_k(x) = -0.5*||B_k x||^2 + w_k.x + C_k     (C_k absorbs logdet, log w, mu-term)
  S_k(x)   = ||B_k x||^2 - 2 w_k.x - 2(C_k - m0)   -> wlp - m0 = -S/2
  out      = sum_x [ m0 + log sum_k exp(-S_k/2) ]

Device dataflow (per core, 25088 padded samples = 196 tiles of 128, grouped
in 8-tile blocks):
  Host ships xall [65, 25088] fp16 (x^T plus a ones row) - no PE transposes.
  Per tile: ONE stationary load (xall column slice [65,128]); matmuls stream
  bmov [65, 1040] fp16 = [B-blocks | -2w/-2C] giving yp [128,1024] f32 and a
  16-wide lp slot (8 consecutive tiles share one PSUM lp bank).  ACT squares
  yp into the 65-strided slots of an 8-tile mega buffer.  Once per 8-tile
  group, ACT copies the whole lp bank [128,128] into the 65th slots, and DVE
  runs ONE grouped reduce [128, 8*16, 65] -> S f32.  Batched phase 2:
  exp(-S/2) on ACT, component-sum + ln + masked accumulate, ones-matmul
  folds partitions; host adds 25000*m0 per core and sums cores.
"""

import numpy as np

N_COMPONENTS = 16
N_FEATURES = 64
N_SAMPLES = 200000
N_CORES = 8
PER_CORE = N_SAMPLES // N_CORES          # 25000
TILE_P = 128
N_TILES = -(-PER_CORE // TILE_P)         # 196
PADDED = N_TILES * TILE_P                # 25088
KD = N_COMPONENTS * N_FEATURES           # 1024
GW = N_FEATURES + 1                      # 65: squares + lp slot per component
GRP = 8                                  # tiles per reduce/evac group
N_GRP = -(-N_TILES // GRP)               # 25 (24 full + one of 4)

_CACHE = {}


def _build_nc():
    import concourse.tile as tile
    from concourse import bacc, mybir

    f32 = mybir.dt.float32
    f16 = mybir.dt.float16

    nc = bacc.Bacc("TRN2", target_bir_lowering=False, debug=False,
                   num_devices=N_CORES)

    xall = nc.dram_tensor("xall", [GW, PADDED], f16, kind="ExternalInput").ap()
    bmov = nc.dram_tensor("bmov", [GW, KD + N_COMPONENTS], f16,
                          kind="ExternalInput").ap()
    mask = nc.dram_tensor("mask", [128, N_TILES], f32, kind="ExternalInput").ap()
    ones = nc.dram_tensor("ones", [128, 1], f32, kind="ExternalInput").ap()
    out = nc.dram_tensor("out", [1, 1], f32, kind="ExternalOutput").ap()

    n_chunks = 8
    chunk = PADDED // n_chunks            # 3136 cols

    with tile.TileContext(nc) as tc:
        with (
            tc.tile_pool(name="const", bufs=1) as const_pool,
            tc.tile_pool(name="ysq", bufs=2) as ysq_pool,
            tc.tile_pool(name="yp", bufs=2, space="PSUM") as yp_pool,
            tc.tile_pool(name="lp", bufs=1, space="PSUM") as lp_pool,
            tc.tile_pool(name="rp", bufs=1, space="PSUM") as rp_pool,
        ):
            xs = const_pool.tile([GW, PADDED], f16)
            dma_engines = [nc.sync, nc.gpsimd, nc.sync, nc.gpsimd]
            for c in range(n_chunks):
                eng = dma_engines[c % len(dma_engines)]
                sl = slice(c * chunk, (c + 1) * chunk)
                eng.dma_start(xs[:, sl], xall[:, sl])
            bm = const_pool.tile([GW, KD + N_COMPONENTS], f16)
            nc.sync.dma_start(bm[:], bmov[:])
            msks = const_pool.tile([128, N_TILES], f32)
            nc.sync.dma_start(msks[:], mask[:])
            on1 = const_pool.tile([128, 1], f32)
            nc.sync.dma_start(on1[:], ones[:])

            sbuf_S = const_pool.tile([128, N_TILES * N_COMPONENTS], f32)

            # two lp PSUM banks; 8-tile group g -> bank g%2, slot j*16
            lp_banks = []
            for b in range(2):
                lpb = lp_pool.tile([128, 512], f32, tag=f"lpb{b}", name=f"lpb{b}")
                lp_banks.append(lpb)

            for g in range(N_GRP):
                gsz = min(GRP, N_TILES - g * GRP)
                ysq = ysq_pool.tile([128, GRP * KD], f16, tag="ysq")
                ysq_v = ysq[:].rearrange("p (t c) -> p t c", c=KD)
                lpe = ysq_pool.tile([128, GRP * N_COMPONENTS], f32, tag="lpe")
                s0 = ysq_pool.tile([128, GRP * KD // 2], f16, tag="s0")
                s1 = ysq_pool.tile([128, GRP * KD // 4], f16, tag="s1")
                lpb = lp_banks[g % 2]
                for j in range(gsz):
                    t = g * GRP + j
                    lhs = xs[:, t * TILE_P:(t + 1) * TILE_P]
                    yp = yp_pool.tile([128, KD], f32, tag="yp")
                    nc.tensor.matmul(yp[:, 0:512], lhs, bm[:, 0:512])
                    nc.tensor.matmul(yp[:, 512:1024], lhs, bm[:, 512:1024])
                    nc.tensor.matmul(lpb[:, j * 16:(j + 1) * 16], lhs,
                                     bm[:, KD:KD + N_COMPONENTS])
                    nc.scalar.activation(ysq_v[:, j, 0:512], yp[:, 0:512],
                                         mybir.ActivationFunctionType.Square)
                    nc.scalar.activation(ysq_v[:, j, 512:1024], yp[:, 512:1024],
                                         mybir.ActivationFunctionType.Square)
                ng = gsz * N_COMPONENTS
                nc.scalar.copy(lpe[:, 0:ng], lpb[:, 0:ng])
                with nc.allow_low_precision("fp16 square tree; S in f32"):
                    half = 32
                    cur = ysq[:, 0:ng * 64].rearrange("p (q i) -> p q i", i=64)
                    buf = [s0, s1]
                    bi = 0
                    while half >= 1:
                        if half > 1:
                            dst = buf[bi][:, 0:ng * half].rearrange(
                                "p (q i) -> p q i", i=half)
                        else:
                            dst = buf[bi][:, 0:ng].rearrange(
                                "p (q i) -> p q i", i=1)
                        nc.vector.tensor_tensor(
                            dst, cur[:, :, 0:half], cur[:, :, half:2 * half],
                            op=mybir.AluOpType.add)
                        cur = dst
                        bi ^= 1
                        half //= 2
                    nc.vector.tensor_tensor(
                        sbuf_S[:, g * GRP * N_COMPONENTS:
                               g * GRP * N_COMPONENTS + ng],
                        cur[:, :, 0], lpe[:, 0:ng], op=mybir.AluOpType.add)

            # phase 2 (batched)
            ebuf = const_pool.tile([128, N_TILES * N_COMPONENTS], f32)
            nc.scalar.activation(ebuf[:], sbuf_S[:],
                                 mybir.ActivationFunctionType.Exp, scale=-0.5)
            esum = const_pool.tile([128, N_TILES], f32)
            nc.vector.reduce_sum(
                esum[:], ebuf[:].rearrange("p (t k) -> p t k", k=N_COMPONENTS),
                axis=mybir.AxisListType.X)
            lnr = const_pool.tile([128, N_TILES], f32)
            nc.scalar.activation(lnr[:], esum[:],
                                 mybir.ActivationFunctionType.Ln)
            msum = const_pool.tile([128, N_TILES], f32)
            nc.vector.tensor_tensor(msum[:], lnr[:], msks[:],
                                    op=mybir.AluOpType.mult)
            csum = const_pool.tile([128, 1], f32)
            nc.vector.reduce_sum(csum[:], msum[:], axis=mybir.AxisListType.X)

            rp = rp_pool.tile([1, 1], f32, tag="rp")
            nc.tensor.matmul(rp[:], on1[:], csum[:])
            res = const_pool.tile([1, 1], f32)
            nc.scalar.copy(res[:], rp[:])
            nc.sync.dma_start(out[:], res[:])

    nc.compile()
    return nc


def _precompute(weights, means, covariances):
    """Host-side O(K d^3) prep in float64. Returns (bmov, m0)."""
    K, d = means.shape
    L = np.linalg.cholesky(covariances.astype(np.float64))
    half_logdet = np.log(np.diagonal(L, axis1=-2, axis2=-1)).sum(-1)
    eye = np.eye(d)
    B = np.stack([np.linalg.solve(L[k], eye) for k in range(K)])  # L^-1
    mu = means.astype(np.float64)
    c = np.einsum('kij,kj->ki', B, mu)
    w_lin = np.einsum('kij,ki->kj', B, c)
    r = (c * c).sum(-1)
    const = (np.log(weights.astype(np.float64))
             - 0.5 * d * np.log(2.0 * np.pi) - half_logdet)
    C = const - 0.5 * r
    m0 = float(C.max()) - 20.0

    bmov = np.zeros((GW, KD + N_COMPONENTS), np.float64)
    for k in range(K):
        bmov[0:d, k * d:(k + 1) * d] = B[k].T
    bmov[0:d, KD:] = (-2.0 * w_lin).T
    bmov[d, KD:] = -2.0 * (C - m0)
    return bmov.astype(np.float16), m0


def _make_inputs(data, bmov):
    mask = np.zeros((128, N_TILES), np.float32)
    for t in range(N_TILES):
        v = min(max(PER_CORE - t * TILE_P, 0), TILE_P)
        mask[:v, t] = 1.0
    ones = np.ones((128, 1), np.float32)

    d16 = data.astype(np.float16)
    in_maps = []
    for c in range(N_CORES):
        sl = d16[c * PER_CORE:(c + 1) * PER_CORE]
        xall = np.zeros((GW, PADDED), np.float16)
        xall[0:N_FEATURES, 0:PER_CORE] = sl.T
        xall[N_FEATURES, :] = 1.0
        in_maps.append({"xall": xall, "bmov": bmov, "mask": mask,
                        "ones": ones})
    return in_maps


def _run(data, weights, means, covariances, trace=False):
    from concourse.bass_utils import run_bass_kernel_spmd

    data = np.asarray(data, np.float32)
    bmov, m0 = _precompute(np.asarray(weights), np.asarray(means),
                           np.asarray(covariances))
    if "nc" not in _CACHE:
        _CACHE["nc"] = _build_nc()
    nc = _CACHE["nc"]

    in_maps = _make_inputs(data, bmov)
    res = run_bass_kernel_spmd(nc, in_maps, list(range(N_CORES)), trace=trace)
    total = 0.0
    for c in range(N_CORES):
        total += float(res.results[c]["out"][0, 0]) + PER_CORE * m0
    return np.float32(total), res


def kernel(data, weights, means, covariances):
    return _run(data, weights, means, covariances)[0]


# revision 16
# speedup vs baseline: 1.3007x; 1.3007x over previous
"""GMM log-likelihood kernel for Trainium2 (Bass/Tile), 8-core data-parallel.

v3 design. Math (host precompute in f64):
  B_k = L_k^{-1},  w_k = B_k^T B_k mu_k
  wlp_k(x) = -0.5*||B_k x||^2 + w_k.x + C_k     (C_k absorbs logdet, log w, mu-term)
  S_k(x)   = ||B_k x||^2 - 2 w_k.x - 2(C_k - m0)   -> wlp - m0 = -S/2
  out      = sum_x [ m0 + log sum_k exp(-S_k/2) ]

Device dataflow (per core, 25088 padded samples = 196 tiles of 128, grouped
in 8-tile blocks):
  Host ships xall [65, 25088] fp16 (x^T plus a ones row) - no PE transposes.
  Per tile: ONE stationary load (xall column slice [65,128]); matmuls stream
  bmov [65, 1040] fp16 = [B-blocks | -2w/-2C] giving yp [128,1024] f32 and a
  16-wide lp slot (8 consecutive tiles share one PSUM lp bank).  ACT squares
  yp into the 65-strided slots of an 8-tile mega buffer.  Once per 8-tile
  group, ACT copies the whole lp bank [128,128] into the 65th slots, and DVE
  runs ONE grouped reduce [128, 8*16, 65] -> S f32.  Batched phase 2:
  exp(-S/2) on ACT, component-sum + ln + masked accumulate, ones-matmul
  folds partitions; host adds 25000*m0 per core and sums cores.
"""

import numpy as np

N_COMPONENTS = 16
N_FEATURES = 64
N_SAMPLES = 200000
N_CORES = 8
PER_CORE = N_SAMPLES // N_CORES          # 25000
TILE_P = 128
N_TILES = -(-PER_CORE // TILE_P)         # 196
PADDED = N_TILES * TILE_P                # 25088
KD = N_COMPONENTS * N_FEATURES           # 1024
GW = N_FEATURES + 1                      # 65: squares + lp slot per component
GRP = 8                                  # tiles per reduce/evac group
N_GRP = -(-N_TILES // GRP)               # 25 (24 full + one of 4)

_CACHE = {}


def _build_nc():
    import concourse.tile as tile
    from concourse import bacc, mybir

    f32 = mybir.dt.float32
    f16 = mybir.dt.float16

    nc = bacc.Bacc("TRN2", target_bir_lowering=False, debug=False,
                   num_devices=N_CORES)

    xall = nc.dram_tensor("xall", [GW, PADDED], f16, kind="ExternalInput").ap()
    bmov = nc.dram_tensor("bmov", [GW, KD + N_COMPONENTS], f16,
                          kind="ExternalInput").ap()
    mask = nc.dram_tensor("mask", [128, N_TILES], f32, kind="ExternalInput").ap()
    ones = nc.dram_tensor("ones", [128, 1], f32, kind="ExternalInput").ap()
    out = nc.dram_tensor("out", [1, 1], f32, kind="ExternalOutput").ap()

    n_chunks = 8
    chunk = PADDED // n_chunks            # 3136 cols

    with tile.TileContext(nc) as tc:
        with (
            tc.tile_pool(name="const", bufs=1) as const_pool,
            tc.tile_pool(name="ysq", bufs=2) as ysq_pool,
            tc.tile_pool(name="yp", bufs=2, space="PSUM") as yp_pool,
            tc.tile_pool(name="lp", bufs=1, space="PSUM") as lp_pool,
            tc.tile_pool(name="rp", bufs=1, space="PSUM") as rp_pool,
        ):
            xs = const_pool.tile([GW, PADDED], f16)
            dma_engines = [nc.sync, nc.gpsimd, nc.sync, nc.gpsimd]
            for c in range(n_chunks):
                eng = dma_engines[c % len(dma_engines)]
                sl = slice(c * chunk, (c + 1) * chunk)
                eng.dma_start(xs[:, sl], xall[:, sl])
            bm = const_pool.tile([GW, KD + N_COMPONENTS], f16)
            nc.sync.dma_start(bm[:], bmov[:])
            msks = const_pool.tile([128, N_TILES], f32)
            nc.sync.dma_start(msks[:], mask[:])
            on1 = const_pool.tile([128, 1], f32)
            nc.sync.dma_start(on1[:], ones[:])

            sbuf_S = const_pool.tile([128, N_TILES * N_COMPONENTS], f32)

            # two lp PSUM banks; 8-tile group g -> bank g%2, slot j*16
            lp_banks = []
            for b in range(2):
                lpb = lp_pool.tile([128, 512], f32, tag=f"lpb{b}", name=f"lpb{b}")
                lp_banks.append(lpb)

            for g in range(N_GRP):
                gsz = min(GRP, N_TILES - g * GRP)
                ysq = ysq_pool.tile([128, GRP * N_COMPONENTS * GW], f16,
                                    tag="ysq")
                ysq_v = ysq[:].rearrange("p (t k i) -> p t k i",
                                         k=N_COMPONENTS, i=GW)
                lpb = lp_banks[g % 2]
                for j in range(gsz):
                    t = g * GRP + j
                    lhs = xs[:, t * TILE_P:(t + 1) * TILE_P]
                    yp = yp_pool.tile([128, KD], f32, tag="yp")
                    nc.tensor.matmul(yp[:, 0:512], lhs, bm[:, 0:512])
                    nc.tensor.matmul(yp[:, 512:1024], lhs, bm[:, 512:1024])
                    nc.tensor.matmul(lpb[:, j * 16:(j + 1) * 16], lhs,
                                     bm[:, KD:KD + N_COMPONENTS])
                    nc.scalar.activation(ysq_v[:, j, :, 0:64], yp[:],
                                         mybir.ActivationFunctionType.Square)
                # batched lp evac into the 65th slots
                nc.scalar.copy(
                    ysq_v[:, 0:gsz, :, 64:65],
                    lpb[:, 0:gsz * 16].rearrange("p (t k i) -> p t k i",
                                                 k=N_COMPONENTS, i=1))
                # one grouped reduce for the whole group
                nc.vector.reduce_sum(
                    sbuf_S[:, g * GRP * N_COMPONENTS:
                           (g * GRP + gsz) * N_COMPONENTS],
                    ysq_v[:, 0:gsz], axis=mybir.AxisListType.X)

            # phase 2 (batched)
            ebuf = const_pool.tile([128, N_TILES * N_COMPONENTS], f32)
            nc.scalar.activation(ebuf[:], sbuf_S[:],
                                 mybir.ActivationFunctionType.Exp, scale=-0.5)
            esum = const_pool.tile([128, N_TILES], f32)
            nc.vector.reduce_sum(
                esum[:], ebuf[:].rearrange("p (t k) -> p t k", k=N_COMPONENTS),
                axis=mybir.AxisListType.X)
            lnr = const_pool.tile([128, N_TILES], f32)
            nc.scalar.activation(lnr[:], esum[:],
                                 mybir.ActivationFunctionType.Ln)
            msum = const_pool.tile([128, N_TILES], f32)
            nc.vector.tensor_tensor(msum[:], lnr[:], msks[:],
                                    op=mybir.AluOpType.mult)
            csum = const_pool.tile([128, 1], f32)
            nc.vector.reduce_sum(csum[:], msum[:], axis=mybir.AxisListType.X)

            rp = rp_pool.tile([1, 1], f32, tag="rp")
            nc.tensor.matmul(rp[:], on1[:], csum[:])
            res = const_pool.tile([1, 1], f32)
            nc.scalar.copy(res[:], rp[:])
            nc.sync.dma_start(out[:], res[:])

    nc.compile()
    return nc


def _precompute(weights, means, covariances):
    """Host-side O(K d^3) prep in float64. Returns (bmov, m0)."""
    K, d = means.shape
    L = np.linalg.cholesky(covariances.astype(np.float64))
    half_logdet = np.log(np.diagonal(L, axis1=-2, axis2=-1)).sum(-1)
    eye = np.eye(d)
    B = np.stack([np.linalg.solve(L[k], eye) for k in range(K)])  # L^-1
    mu = means.astype(np.float64)
    c = np.einsum('kij,kj->ki', B, mu)
    w_lin = np.einsum('kij,ki->kj', B, c)
    r = (c * c).sum(-1)
    const = (np.log(weights.astype(np.float64))
             - 0.5 * d * np.log(2.0 * np.pi) - half_logdet)
    C = const - 0.5 * r
    m0 = float(C.max()) - 20.0

    bmov = np.zeros((GW, KD + N_COMPONENTS), np.float64)
    for k in range(K):
        bmov[0:d, k * d:(k + 1) * d] = B[k].T
    bmov[0:d, KD:] = (-2.0 * w_lin).T
    bmov[d, KD:] = -2.0 * (C - m0)
    return bmov.astype(np.float16), m0


def _make_inputs(data, bmov):
    mask = np.zeros((128, N_TILES), np.float32)
    for t in range(N_TILES):
        v = min(max(PER_CORE - t * TILE_P, 0), TILE_P)
        mask[:v, t] = 1.0
    ones = np.ones((128, 1), np.float32)

    d16 = data.astype(np.float16)
    in_maps = []
    for c in range(N_CORES):
        sl = d16[c * PER_CORE:(c + 1) * PER_CORE]
        xall = np.zeros((GW, PADDED), np.float16)
        xall[0:N_FEATURES, 0:PER_CORE] = sl.T
        xall[N_FEATURES, :] = 1.0
        in_maps.append({"xall": xall, "bmov": bmov, "mask": mask,
                        "ones": ones})
    return in_maps


def _run(data, weights, means, covariances, trace=False):
    from concourse.bass_utils import run_bass_kernel_spmd

    data = np.asarray(data, np.float32)
    bmov, m0 = _precompute(np.asarray(weights), np.asarray(means),
                           np.asarray(covariances))
    if "nc" not in _CACHE:
        _CACHE["nc"] = _build_nc()
    nc = _CACHE["nc"]

    in_maps = _make_inputs(data, bmov)
    res = run_bass_kernel_spmd(nc, in_maps, list(range(N_CORES)), trace=trace)
    total = 0.0
    for c in range(N_CORES):
        total += float(res.results[c]["out"][0, 0]) + PER_CORE * m0
    return np.float32(total), res


def kernel(data, weights, means, covariances):
    return _run(data, weights, means, covariances)[0]


# revision 21
# speedup vs baseline: 1.4050x; 1.0801x over previous
"""GMM log-likelihood kernel for Trainium2 (Bass/Tile), 8-core data-parallel.

v3 design. Math (host precompute in f64):
  B_k = L_k^{-1},  w_k = B_k^T B_k mu_k
  wlp_k(x) = -0.5*||B_k x||^2 + w_k.x + C_k     (C_k absorbs logdet, log w, mu-term)
  S_k(x)   = ||B_k x||^2 - 2 w_k.x - 2(C_k - m0)   -> wlp - m0 = -S/2
  out      = sum_x [ m0 + log sum_k exp(-S_k/2) ]

Device dataflow (per core, 25088 padded samples = 196 tiles of 128, grouped
in 8-tile blocks):
  Host ships xall [65, 25088] fp16 (x^T plus a ones row) - no PE transposes.
  Per tile: ONE stationary load (xall column slice [65,128]); matmuls stream
  bmov [65, 1040] fp16 = [B-blocks | -2w/-2C] giving yp [128,1024] f32 and a
  16-wide lp slot (8 consecutive tiles share one PSUM lp bank).  ACT squares
  yp into the 65-strided slots of an 8-tile mega buffer.  Once per 8-tile
  group, ACT copies the whole lp bank [128,128] into the 65th slots, and DVE
  runs ONE grouped reduce [128, 8*16, 65] -> S f32.  Batched phase 2:
  exp(-S/2) on ACT, component-sum + ln + masked accumulate, ones-matmul
  folds partitions; host adds 25000*m0 per core and sums cores.
"""

import numpy as np

N_COMPONENTS = 16
N_FEATURES = 64
N_SAMPLES = 200000
N_CORES = 8
PER_CORE = N_SAMPLES // N_CORES          # 25000
TILE_P = 128
N_TILES = -(-PER_CORE // TILE_P)         # 196
PADDED = N_TILES * TILE_P                # 25088
KD = N_COMPONENTS * N_FEATURES           # 1024
GW = N_FEATURES + 1                      # 65: squares + lp slot per component
GRP = 8                                  # tiles per reduce/evac group
N_GRP = -(-N_TILES // GRP)               # 25 (24 full + one of 4)
HALF_G = 13                              # issue phase2 first half after this group
HALF_T = (HALF_G + 1) * GRP              # 112 tiles

_CACHE = {}


def _build_nc():
    import concourse.tile as tile
    from concourse import bacc, mybir

    f32 = mybir.dt.float32
    f16 = mybir.dt.float16

    nc = bacc.Bacc("TRN2", target_bir_lowering=False, debug=False,
                   num_devices=N_CORES)

    xall = nc.dram_tensor("xall", [GW, PADDED], f16, kind="ExternalInput").ap()
    bmov = nc.dram_tensor("bmov", [GW, KD + N_COMPONENTS], f16,
                          kind="ExternalInput").ap()
    mask = nc.dram_tensor("mask", [128, N_TILES], f32, kind="ExternalInput").ap()
    ones = nc.dram_tensor("ones", [128, 1], f32, kind="ExternalInput").ap()
    out = nc.dram_tensor("out", [1, 1], f32, kind="ExternalOutput").ap()

    n_chunks = 8
    chunk = PADDED // n_chunks            # 3136 cols

    with tile.TileContext(nc) as tc:
        with (
            tc.tile_pool(name="const", bufs=1) as const_pool,
            tc.tile_pool(name="ysq", bufs=2) as ysq_pool,
            tc.tile_pool(name="yp", bufs=2, space="PSUM") as yp_pool,
            tc.tile_pool(name="lp", bufs=1, space="PSUM") as lp_pool,
            tc.tile_pool(name="rp", bufs=1, space="PSUM") as rp_pool,
        ):
            bm = const_pool.tile([GW, KD + N_COMPONENTS], f16)
            nc.sync.dma_start(bm[:], bmov[:])
            msks = const_pool.tile([128, N_TILES], f32)
            nc.gpsimd.dma_start(msks[:], mask[:])
            on1 = const_pool.tile([128, 1], f32)
            nc.gpsimd.dma_start(on1[:], ones[:])
            xs = const_pool.tile([GW, PADDED], f16)
            # progressive chunks (in tiles) so tile 0 lands fast
            sizes = [4, 8, 12, 16, 24, 24, 32, 36, 40]
            dma_engines = [nc.sync, nc.gpsimd]
            pos = 0
            for c, sz in enumerate(sizes):
                eng = dma_engines[c % 2]
                sl = slice(pos * TILE_P, (pos + sz) * TILE_P)
                eng.dma_start(xs[:, sl], xall[:, sl])
                pos += sz
            assert pos == N_TILES

            sbuf_S = const_pool.tile([128, N_TILES * N_COMPONENTS], f32)
            ebuf = const_pool.tile([128, N_TILES * N_COMPONENTS], f32)
            esum = const_pool.tile([128, N_TILES], f32)

            # two lp PSUM banks; 8-tile group g -> bank g%2, slot j*16
            lp_banks = []
            for b in range(2):
                lpb = lp_pool.tile([128, 512], f32, tag=f"lpb{b}", name=f"lpb{b}")
                lp_banks.append(lpb)

            gsizes = [2, 2, 4] + [8] * 23 + [4]
            assert sum(gsizes) == N_TILES
            cuts = {112: False, 176: False}
            tbase = 0
            for g, gsz in enumerate(gsizes):
                ysq = ysq_pool.tile([128, GRP * N_COMPONENTS * GW], f16,
                                    tag="ysq")
                ysq_v = ysq[:].rearrange("p (t k i) -> p t k i",
                                         k=N_COMPONENTS, i=GW)
                lpb = lp_banks[g % 2]
                for j in range(gsz):
                    t = tbase + j
                    lhs = xs[:, t * TILE_P:(t + 1) * TILE_P]
                    yp = yp_pool.tile([128, KD], f32, tag="yp")
                    nc.tensor.matmul(yp[:, 0:512], lhs, bm[:, 0:512])
                    nc.tensor.matmul(yp[:, 512:1024], lhs, bm[:, 512:1024])
                    nc.tensor.matmul(lpb[:, j * 16:(j + 1) * 16], lhs,
                                     bm[:, KD:KD + N_COMPONENTS])
                    nc.scalar.activation(ysq_v[:, j, :, 0:64], yp[:],
                                         mybir.ActivationFunctionType.Square)
                # batched lp evac into the 65th slots
                nc.scalar.copy(
                    ysq_v[:, 0:gsz, :, 64:65],
                    lpb[:, 0:gsz * 16].rearrange("p (t k i) -> p t k i",
                                                 k=N_COMPONENTS, i=1))
                # one grouped reduce for the whole group
                nc.vector.reduce_sum(
                    sbuf_S[:, tbase * N_COMPONENTS:
                           (tbase + gsz) * N_COMPONENTS],
                    ysq_v[:, 0:gsz], axis=mybir.AxisListType.X)
                tbase += gsz
                for cut in cuts:
                    if not cuts[cut] and tbase >= cut:
                        cuts[cut] = True
                        t0 = {112: 0, 176: 112}[cut]
                        c0, c1 = t0 * N_COMPONENTS, cut * N_COMPONENTS
                        nc.scalar.activation(
                            ebuf[:, c0:c1], sbuf_S[:, c0:c1],
                            mybir.ActivationFunctionType.Exp, scale=-0.5)
                        nc.vector.reduce_sum(
                            esum[:, t0:cut],
                            ebuf[:, c0:c1].rearrange("p (t k) -> p t k",
                                                     k=N_COMPONENTS),
                            axis=mybir.AxisListType.X)

            # phase 2 (last slice; earlier slices issued mid-loop)
            h = 176 * N_COMPONENTS
            nc.scalar.activation(ebuf[:, h:], sbuf_S[:, h:],
                                 mybir.ActivationFunctionType.Exp, scale=-0.5)
            nc.vector.reduce_sum(
                esum[:, 176:],
                ebuf[:, h:].rearrange("p (t k) -> p t k", k=N_COMPONENTS),
                axis=mybir.AxisListType.X)
            lnr = const_pool.tile([128, N_TILES], f32)
            nc.scalar.activation(lnr[:], esum[:],
                                 mybir.ActivationFunctionType.Ln)
            msum = const_pool.tile([128, N_TILES], f32)
            nc.vector.tensor_tensor(msum[:], lnr[:], msks[:],
                                    op=mybir.AluOpType.mult)
            csum = const_pool.tile([128, 1], f32)
            nc.vector.reduce_sum(csum[:], msum[:], axis=mybir.AxisListType.X)

            rp = rp_pool.tile([1, 1], f32, tag="rp")
            nc.tensor.matmul(rp[:], on1[:], csum[:])
            res = const_pool.tile([1, 1], f32)
            nc.scalar.copy(res[:], rp[:])
            nc.sync.dma_start(out[:], res[:])

    nc.compile()
    return nc


def _precompute(weights, means, covariances):
    """Host-side O(K d^3) prep in float64. Returns (bmov, m0)."""
    K, d = means.shape
    L = np.linalg.cholesky(covariances.astype(np.float64))
    half_logdet = np.log(np.diagonal(L, axis1=-2, axis2=-1)).sum(-1)
    eye = np.eye(d)
    B = np.stack([np.linalg.solve(L[k], eye) for k in range(K)])  # L^-1
    mu = means.astype(np.float64)
    c = np.einsum('kij,kj->ki', B, mu)
    w_lin = np.einsum('kij,ki->kj', B, c)
    r = (c * c).sum(-1)
    const = (np.log(weights.astype(np.float64))
             - 0.5 * d * np.log(2.0 * np.pi) - half_logdet)
    C = const - 0.5 * r
    m0 = float(C.max()) - 20.0

    bmov = np.zeros((GW, KD + N_COMPONENTS), np.float64)
    for k in range(K):
        bmov[0:d, k * d:(k + 1) * d] = B[k].T
    bmov[0:d, KD:] = (-2.0 * w_lin).T
    bmov[d, KD:] = -2.0 * (C - m0)
    return bmov.astype(np.float16), m0


def _make_inputs(data, bmov):
    mask = np.zeros((128, N_TILES), np.float32)
    for t in range(N_TILES):
        v = min(max(PER_CORE - t * TILE_P, 0), TILE_P)
        mask[:v, t] = 1.0
    ones = np.ones((128, 1), np.float32)

    d16 = data.astype(np.float16)
    in_maps = []
    for c in range(N_CORES):
        sl = d16[c * PER_CORE:(c + 1) * PER_CORE]
        xall = np.zeros((GW, PADDED), np.float16)
        xall[0:N_FEATURES, 0:PER_CORE] = sl.T
        xall[N_FEATURES, :] = 1.0
        in_maps.append({"xall": xall, "bmov": bmov, "mask": mask,
                        "ones": ones})
    return in_maps


def _run(data, weights, means, covariances, trace=False):
    from concourse.bass_utils import run_bass_kernel_spmd

    data = np.asarray(data, np.float32)
    bmov, m0 = _precompute(np.asarray(weights), np.asarray(means),
                           np.asarray(covariances))
    if "nc" not in _CACHE:
        _CACHE["nc"] = _build_nc()
    nc = _CACHE["nc"]

    in_maps = _make_inputs(data, bmov)
    res = run_bass_kernel_spmd(nc, in_maps, list(range(N_CORES)), trace=trace)
    total = 0.0
    for c in range(N_CORES):
        total += float(res.results[c]["out"][0, 0]) + PER_CORE * m0
    return np.float32(total), res


def kernel(data, weights, means, covariances):
    return _run(data, weights, means, covariances)[0]


# revision 22
# speedup vs baseline: 1.4095x; 1.0032x over previous
"""GMM log-likelihood kernel for Trainium2 (Bass/Tile), 8-core data-parallel.

v3 design. Math (host precompute in f64):
  B_k = L_k^{-1},  w_k = B_k^T B_k mu_k
  wlp_k(x) = -0.5*||B_k x||^2 + w_k.x + C_k     (C_k absorbs logdet, log w, mu-term)
  S_k(x)   = ||B_k x||^2 - 2 w_k.x - 2(C_k - m0)   -> wlp - m0 = -S/2
  out      = sum_x [ m0 + log sum_k exp(-S_k/2) ]

Device dataflow (per core, 25088 padded samples = 196 tiles of 128, grouped
in 8-tile blocks):
  Host ships xall [65, 25088] fp16 (x^T plus a ones row) - no PE transposes.
  Per tile: ONE stationary load (xall column slice [65,128]); matmuls stream
  bmov [65, 1040] fp16 = [B-blocks | -2w/-2C] giving yp [128,1024] f32 and a
  16-wide lp slot (8 consecutive tiles share one PSUM lp bank).  ACT squares
  yp into the 65-strided slots of an 8-tile mega buffer.  Once per 8-tile
  group, ACT copies the whole lp bank [128,128] into the 65th slots, and DVE
  runs ONE grouped reduce [128, 8*16, 65] -> S f32.  Batched phase 2:
  exp(-S/2) on ACT, component-sum + ln + masked accumulate, ones-matmul
  folds partitions; host adds 25000*m0 per core and sums cores.
"""

import numpy as np

N_COMPONENTS = 16
N_FEATURES = 64
N_SAMPLES = 200000
N_CORES = 8
PER_CORE = N_SAMPLES // N_CORES          # 25000
TILE_P = 128
N_TILES = -(-PER_CORE // TILE_P)         # 196
PADDED = N_TILES * TILE_P                # 25088
KD = N_COMPONENTS * N_FEATURES           # 1024
GW = N_FEATURES + 1                      # 65: squares + lp slot per component
GRP = 8                                  # tiles per reduce/evac group
N_GRP = -(-N_TILES // GRP)               # 25 (24 full + one of 4)
HALF_G = 13                              # issue phase2 first half after this group
HALF_T = (HALF_G + 1) * GRP              # 112 tiles

_CACHE = {}


def _build_nc():
    import concourse.tile as tile
    from concourse import bacc, mybir

    f32 = mybir.dt.float32
    f16 = mybir.dt.float16

    nc = bacc.Bacc("TRN2", target_bir_lowering=False, debug=False,
                   num_devices=N_CORES)

    xall = nc.dram_tensor("xall", [GW, PADDED], f16, kind="ExternalInput").ap()
    bmov = nc.dram_tensor("bmov", [GW, KD + N_COMPONENTS], f16,
                          kind="ExternalInput").ap()
    mask = nc.dram_tensor("mask", [128, N_TILES], f32, kind="ExternalInput").ap()
    ones = nc.dram_tensor("ones", [128, 1], f32, kind="ExternalInput").ap()
    out = nc.dram_tensor("out", [1, 1], f32, kind="ExternalOutput").ap()

    n_chunks = 8
    chunk = PADDED // n_chunks            # 3136 cols

    with tile.TileContext(nc) as tc:
        with (
            tc.tile_pool(name="const", bufs=1) as const_pool,
            tc.tile_pool(name="ysq", bufs=3) as ysq_pool,
            tc.tile_pool(name="yp", bufs=2, space="PSUM") as yp_pool,
            tc.tile_pool(name="lp", bufs=1, space="PSUM") as lp_pool,
            tc.tile_pool(name="rp", bufs=1, space="PSUM") as rp_pool,
        ):
            bm = const_pool.tile([GW, KD + N_COMPONENTS], f16)
            nc.sync.dma_start(bm[:], bmov[:])
            msks = const_pool.tile([128, N_TILES], f32)
            nc.gpsimd.dma_start(msks[:], mask[:])
            on1 = const_pool.tile([128, 1], f32)
            nc.gpsimd.dma_start(on1[:], ones[:])
            xs = const_pool.tile([GW, PADDED], f16)
            # progressive chunks (in tiles) so tile 0 lands fast
            sizes = [4, 8, 12, 16, 24, 24, 32, 36, 40]
            dma_engines = [nc.sync, nc.gpsimd]
            pos = 0
            for c, sz in enumerate(sizes):
                eng = dma_engines[c % 2]
                sl = slice(pos * TILE_P, (pos + sz) * TILE_P)
                eng.dma_start(xs[:, sl], xall[:, sl])
                pos += sz
            assert pos == N_TILES

            sbuf_S = const_pool.tile([128, N_TILES * N_COMPONENTS], f32)
            ebuf = const_pool.tile([128, N_TILES * N_COMPONENTS], f32)
            esum = const_pool.tile([128, N_TILES], f32)

            # two lp PSUM banks; 8-tile group g -> bank g%2, slot j*16
            lp_banks = []
            for b in range(2):
                lpb = lp_pool.tile([128, 512], f32, tag=f"lpb{b}", name=f"lpb{b}")
                lp_banks.append(lpb)

            gsizes = [2, 2, 4] + [8] * 23 + [4]
            assert sum(gsizes) == N_TILES
            cuts = {112: False, 176: False}
            tbase = 0
            for g, gsz in enumerate(gsizes):
                ysq = ysq_pool.tile([128, GRP * N_COMPONENTS * GW], f16,
                                    tag="ysq")
                ysq_v = ysq[:].rearrange("p (t k i) -> p t k i",
                                         k=N_COMPONENTS, i=GW)
                lpb = lp_banks[g % 2]
                for j in range(gsz):
                    t = tbase + j
                    lhs = xs[:, t * TILE_P:(t + 1) * TILE_P]
                    yp = yp_pool.tile([128, KD], f32, tag="yp")
                    nc.tensor.matmul(yp[:, 0:512], lhs, bm[:, 0:512])
                    nc.tensor.matmul(yp[:, 512:1024], lhs, bm[:, 512:1024])
                    nc.tensor.matmul(lpb[:, j * 16:(j + 1) * 16], lhs,
                                     bm[:, KD:KD + N_COMPONENTS])
                    nc.scalar.activation(ysq_v[:, j, :, 0:64], yp[:],
                                         mybir.ActivationFunctionType.Square)
                # batched lp evac into the 65th slots
                nc.scalar.copy(
                    ysq_v[:, 0:gsz, :, 64:65],
                    lpb[:, 0:gsz * 16].rearrange("p (t k i) -> p t k i",
                                                 k=N_COMPONENTS, i=1))
                # one grouped reduce for the whole group
                nc.vector.reduce_sum(
                    sbuf_S[:, tbase * N_COMPONENTS:
                           (tbase + gsz) * N_COMPONENTS],
                    ysq_v[:, 0:gsz], axis=mybir.AxisListType.X)
                tbase += gsz
                for cut in cuts:
                    if not cuts[cut] and tbase >= cut:
                        cuts[cut] = True
                        t0 = {112: 0, 176: 112}[cut]
                        c0, c1 = t0 * N_COMPONENTS, cut * N_COMPONENTS
                        nc.scalar.activation(
                            ebuf[:, c0:c1], sbuf_S[:, c0:c1],
                            mybir.ActivationFunctionType.Exp, scale=-0.5)
                        nc.vector.reduce_sum(
                            esum[:, t0:cut],
                            ebuf[:, c0:c1].rearrange("p (t k) -> p t k",
                                                     k=N_COMPONENTS),
                            axis=mybir.AxisListType.X)

            # phase 2 (last slice; earlier slices issued mid-loop)
            h = 176 * N_COMPONENTS
            nc.scalar.activation(ebuf[:, h:], sbuf_S[:, h:],
                                 mybir.ActivationFunctionType.Exp, scale=-0.5)
            nc.vector.reduce_sum(
                esum[:, 176:],
                ebuf[:, h:].rearrange("p (t k) -> p t k", k=N_COMPONENTS),
                axis=mybir.AxisListType.X)
            lnr = const_pool.tile([128, N_TILES], f32)
            nc.scalar.activation(lnr[:], esum[:],
                                 mybir.ActivationFunctionType.Ln)
            msum = const_pool.tile([128, N_TILES], f32)
            nc.vector.tensor_tensor(msum[:], lnr[:], msks[:],
                                    op=mybir.AluOpType.mult)
            csum = const_pool.tile([128, 1], f32)
            nc.vector.reduce_sum(csum[:], msum[:], axis=mybir.AxisListType.X)

            rp = rp_pool.tile([1, 1], f32, tag="rp")
            nc.tensor.matmul(rp[:], on1[:], csum[:])
            res = const_pool.tile([1, 1], f32)
            nc.scalar.copy(res[:], rp[:])
            nc.sync.dma_start(out[:], res[:])

    nc.compile()
    return nc


def _precompute(weights, means, covariances):
    """Host-side O(K d^3) prep in float64. Returns (bmov, m0)."""
    K, d = means.shape
    L = np.linalg.cholesky(covariances.astype(np.float64))
    half_logdet = np.log(np.diagonal(L, axis1=-2, axis2=-1)).sum(-1)
    eye = np.eye(d)
    B = np.stack([np.linalg.solve(L[k], eye) for k in range(K)])  # L^-1
    mu = means.astype(np.float64)
    c = np.einsum('kij,kj->ki', B, mu)
    w_lin = np.einsum('kij,ki->kj', B, c)
    r = (c * c).sum(-1)
    const = (np.log(weights.astype(np.float64))
             - 0.5 * d * np.log(2.0 * np.pi) - half_logdet)
    C = const - 0.5 * r
    m0 = float(C.max()) - 20.0

    bmov = np.zeros((GW, KD + N_COMPONENTS), np.float64)
    for k in range(K):
        bmov[0:d, k * d:(k + 1) * d] = B[k].T
    bmov[0:d, KD:] = (-2.0 * w_lin).T
    bmov[d, KD:] = -2.0 * (C - m0)
    return bmov.astype(np.float16), m0


def _make_inputs(data, bmov):
    mask = np.zeros((128, N_TILES), np.float32)
    for t in range(N_TILES):
        v = min(max(PER_CORE - t * TILE_P, 0), TILE_P)
        mask[:v, t] = 1.0
    ones = np.ones((128, 1), np.float32)

    d16 = data.astype(np.float16)
    in_maps = []
    for c in range(N_CORES):
        sl = d16[c * PER_CORE:(c + 1) * PER_CORE]
        xall = np.zeros((GW, PADDED), np.float16)
        xall[0:N_FEATURES, 0:PER_CORE] = sl.T
        xall[N_FEATURES, :] = 1.0
        in_maps.append({"xall": xall, "bmov": bmov, "mask": mask,
                        "ones": ones})
    return in_maps


def _run(data, weights, means, covariances, trace=False):
    from concourse.bass_utils import run_bass_kernel_spmd

    data = np.asarray(data, np.float32)
    bmov, m0 = _precompute(np.asarray(weights), np.asarray(means),
                           np.asarray(covariances))
    if "nc" not in _CACHE:
        _CACHE["nc"] = _build_nc()
    nc = _CACHE["nc"]

    in_maps = _make_inputs(data, bmov)
    res = run_bass_kernel_spmd(nc, in_maps, list(range(N_CORES)), trace=trace)
    total = 0.0
    for c in range(N_CORES):
        total += float(res.results[c]["out"][0, 0]) + PER_CORE * m0
    return np.float32(total), res


def kernel(data, weights, means, covariances):
    return _run(data, weights, means, covariances)[0]
